# revision 46
# baseline (speedup 1.0000x reference)
"""Trainium2 Bass kernel for nn_CompositeEmbeddingA (octree composite embedding).

Per sample (1 sample per NeuronCore, batch=8 over 8 cores):
  layers 0-2 (depths 1-3): x = val_emb[v] + pos0[p0] + pos1[p1] + pos2[p2] + dep_emb[d]
  layers 3-4: same sum w/o dep, then Conv1d(E,E,kernel=stride=k), k=4 (l3) / 8 (l4)

Design (v3): every layer is out = MultiHot @ Table on the PE.
  - conv folded into the tables host-side (per tap j, T_j = concat(tables) @ w[:,:,j].T)
  - multi-hot built ON THE HOST, shipped as fp8; PE does only fp8e4m3 DoubleRow
    matmuls (2 chunks of 128 rows per instruction at 0.5 cycles/row); tables are
    scaled hi+lo fp8 pairs (residual correction), eviction rescales by 1/S.
  - L4's multi-hot is arithmetically 2-PACKED: one byte v = a + 2b carries the
    hot bits of a chunk pair. v feeds k-tile 0 directly with table Q(T0); a
    single is_ge op (DVE for pairs 0-2, GPSIMD for 3-5) derives b = (v>=2) for
    k-tile 1 with compensated table U = T1 - 2*Q(T0), which cancels exactly.
  - layers 0+1 (72 tokens) use one precomputed row per token instead of table
    rows (host computes those 72 sums directly).
  - output written as bf16, upcast on the host.
"""

import sys

for _p in ("/opt/trn_rl_repo",):
    if _p not in sys.path:
        sys.path.insert(0, _p)

import numpy as np
import ml_dtypes

E = 256
BATCH = 8
LAYER_SIZES = (8, 64, 512, 4096, 32768)
CONV_SIZE = {3: 4, 4: 8}

F8 = ml_dtypes.float8_e4m3
BF16 = ml_dtypes.bfloat16
F8_MAX = float(ml_dtypes.finfo(F8).max)
_F8_LUT = np.array(
    [np.asarray(float(x), F8).view(np.uint8) for x in range(4)], np.uint8
)

# virtual layers: B = real layers 0-2 merged; L3/L4 conv layers.
#   B: 584 out tokens padded to 640 (5 ttiles); rows = 72 per-token rows
#      (l0+l1) + l2 table (3 val + 189 pos + dep uniques) -> 4 chunks, 2 pairs
#   L3: 1024 tokens (8 tt); 4 taps x 192 rows = 768 -> 6 chunks, 3 pairs
#   L4: 4096 tokens (32 tt); 8 taps x 192 rows = 1536 -> 12 chunks, 6 pairs,
#       multi-hot 2-packed (one 128-col block per pair)
_L = [
    dict(name="B", T=584, Tp=640, ntt=5, nch=4, packed=False),
    dict(name="L3", T=1024, Tp=1024, ntt=8, nch=6, packed=False),
    dict(name="L4", T=4096, Tp=4096, ntt=32, nch=12, packed=True),
]
_mhb = 0
_cb = 0
_orow = 0
for _d in _L:
    _d["mh_base"] = _mhb
    _d["cb"] = _cb
    _d["out_row0"] = _orow
    _d["npairs"] = _d["nch"] // 2
    _d["nblk"] = _d["npairs"] if _d["packed"] else _d["nch"]
    _mhb += _d["ntt"] * _d["nblk"] * 128
    _cb += _d["nch"]
    _orow += _d["Tp"]
NCH = _cb
MH_TOTAL = _mhb  # DRAM mh elements per partition
DRV_BASE = MH_TOTAL  # derived (b) region appended in SBUF, mirrors L4's layout
DRV_TOTAL = _L[2]["ntt"] * _L[2]["npairs"] * 128
OUT_ROWS = _orow  # 5760

# schedule: (layer_index, ttile_start, n_compute_ttiles, mh piece split, store
# split). pieces may cover more ttiles than computed (B loads all 5, computes
# 4; the B tail ttile is the final tiny store, data ready from the start).
SCHEDULE = [
    (2, 0, 8, (1,) * 8, (8,)),
    (2, 8, 8, (1, 1, 2, 2, 2), (8,)),
    (2, 16, 8, (2,) * 4, (8,)),
    (2, 24, 8, (2,) * 4, (8,)),
    (1, 0, 8, (2, 2, 2, 2), (4, 4)),
    (0, 0, 4, (5,), (2, 2)),
    (0, 4, 1, (), (1,)),
]
EVICT_PAT = ("act", "dve", "act", "act")  # DVE also runs derived-stream ops
DVE_PAIRS = 4  # L4 pairs 0..3 derived on DVE, 4..5 on GPSIMD
TBL_AFTER = {2: 2}  # layers whose lo-plane table load defers N mh pieces
TBH_AFTER = {2: 1}  # layers whose hi-plane table load defers N mh pieces
NWARM = 28  # dummy PE matmuls at t=0 so the p-state ramp finishes before real work


def _build_tables(params):
    """Folded f32 tables per virtual layer (core-independent parts)."""
    out = {}
    v2 = np.asarray(params["val_emb_2"], np.float32)[1:4]
    pe2 = np.asarray(params["pos_emb_2"], np.float32)
    out["B_l2"] = np.concatenate([v2, pe2[0][1:64], pe2[1][1:64], pe2[2][1:64]], 0)
    for name, l in (("L3", 3), ("L4", 4)):
        k = CONV_SIZE[l]
        w = np.asarray(params[f"conv_w_{l}"], np.float32)
        b = np.asarray(params[f"conv_b_{l}"], np.float32)
        pe = np.asarray(params[f"pos_emb_{l}"], np.float32)
        base = np.concatenate(
            [
                np.asarray(params[f"val_emb_{l}"], np.float32)[1:4],
                pe[0][1:64],
                pe[1][1:64],
                pe[2][1:64],
            ],
            0,
        )  # [192, E]
        taps = []
        for j in range(k):
            f = base @ w[:, :, j].T
            if j == 0:
                f[:3] += b  # bias fires exactly once per token via the val row
            taps.append(f)
        out[name] = np.concatenate(taps, 0)  # [192k, E]
    return out


def _layer_scale(rows, d):
    """Power-of-2 scale so scaled tables (incl. packed compensation U) fit fp8."""
    nch = d["nch"]
    buf = np.zeros((nch * 128, E), np.float32)
    buf[: rows.shape[0]] = rows
    ch = buf.reshape(nch, 128, E)
    amax = float(np.abs(ch).max())
    if d["packed"]:
        for q in range(d["npairs"]):
            amax = max(amax, float(np.abs(ch[2 * q + 1] - 2.0 * ch[2 * q]).max()))
    return 2.0 ** np.floor(np.log2(0.85 * F8_MAX / amax))


def _pack_layer_tb(rows, d, S):
    """Quantize layer rows -> (hi_plane, lo_plane), each [128, nch*E] fp8."""
    nch = d["nch"]
    buf = np.zeros((nch * 128, E), np.float32)
    buf[: rows.shape[0]] = rows
    ch = buf.reshape(nch, 128, E)
    hi = np.zeros((nch, 128, E), F8)
    lo = np.zeros((nch, 128, E), F8)

    def q(A):
        h = A.astype(F8)
        l = (A - h.astype(np.float32)).astype(F8)
        return h, l

    if not d["packed"]:
        for c in range(nch):
            hi[c], lo[c] = q(ch[c] * S)
    else:
        for qq in range(d["npairs"]):
            h0, l0 = q(ch[2 * qq] * S)
            Q0 = h0.astype(np.float32) + l0.astype(np.float32)
            hu, lu = q(ch[2 * qq + 1] * S - 2.0 * Q0)
            hi[2 * qq], lo[2 * qq] = h0, l0
            hi[2 * qq + 1], lo[2 * qq + 1] = hu, lu
    assert np.isfinite(hi.astype(np.float32)).all()
    assert np.isfinite(lo.astype(np.float32)).all()

    def plane(x):
        return np.ascontiguousarray(x.transpose(1, 0, 2)).reshape(128, nch * E)

    return plane(hi), plane(lo)


def _build_mh(value, depth, position, b, dep2_uniq):
    """Host-built multi-hot for core b: [128, MH_TOTAL] uint8 (fp8 bits)."""
    pieces = []

    def emit(M, d):
        # M: [nch*128, Tp] uint8 hot counts (0/1)
        if d["packed"]:
            Mp = M.reshape(d["npairs"], 2, 128, d["Tp"])
            M = (Mp[:, 0] + 2 * Mp[:, 1]).reshape(d["npairs"] * 128, d["Tp"])
        V = _F8_LUT[M]
        pieces.append(
            V.reshape(d["nblk"], 128, d["ntt"], 128)
            .transpose(1, 2, 0, 3)
            .reshape(128, -1)
        )

    def scatter(r_ids, t_ids, d):
        M = np.zeros(d["nch"] * 128 * d["Tp"], np.uint8)
        M[r_ids * d["Tp"] + t_ids] = 1
        return M.reshape(d["nch"] * 128, d["Tp"])

    # --- B ---
    d = _L[0]
    t01 = np.arange(72)
    v2 = value[b, 72:584]
    p2 = position[b, 72:584]
    d2 = depth[b, 72:584]
    t2 = np.arange(72, 584)
    dep_rows = 264 + np.searchsorted(dep2_uniq, d2)
    r_ids = np.concatenate(
        [
            t01,
            72 + (v2 - 1),
            75 + (p2[:, 0] - 1),
            138 + (p2[:, 1] - 1),
            201 + (p2[:, 2] - 1),
            dep_rows,
        ]
    )
    t_ids = np.concatenate([t01, t2, t2, t2, t2, t2])
    emit(scatter(r_ids, t_ids, d), d)

    # --- conv layers ---
    lo = 584
    for d, l in ((_L[1], 3), (_L[2], 4)):
        k = CONV_SIZE[l]
        T = d["T"]
        v = value[b, lo : lo + T * k].reshape(T, k)
        p = position[b, lo : lo + T * k].reshape(T, k, 3)
        t = np.broadcast_to(np.arange(T)[:, None], (T, k))
        jb = np.broadcast_to(np.arange(k)[None, :] * 192, (T, k))
        r_ids = np.concatenate(
            [
                (jb + v - 1).ravel(),
                (jb + 3 + p[:, :, 0] - 1).ravel(),
                (jb + 66 + p[:, :, 1] - 1).ravel(),
                (jb + 129 + p[:, :, 2] - 1).ravel(),
            ]
        )
        t_ids = np.concatenate([t.ravel()] * 4)
        emit(scatter(r_ids, t_ids, d), d)
        lo += T * k

    return np.concatenate(pieces, axis=1)


_CACHE = {}


def _get_nc(inv_scales):
    key = ("v4.1", tuple(inv_scales), tuple(SCHEDULE), EVICT_PAT, DVE_PAIRS, NWARM)
    if key in _CACHE:
        return _CACHE[key]

    import concourse.bass as bass
    import concourse.tile as tile
    from concourse import bacc, mybir
    from contextlib import ExitStack

    f32 = mybir.dt.float32
    bf16 = mybir.dt.bfloat16
    f8 = mybir.dt.float8e4
    A = mybir.ActivationFunctionType
    DR = mybir.MatmulPerfMode.DoubleRow

    nc = bacc.Bacc(trn_type="TRN2", target_bir_lowering=False, debug=False)
    mh_d = nc.dram_tensor("mh", [128, MH_TOTAL], f8, kind="ExternalInput").ap()
    tb_d = nc.dram_tensor("tb", [128, 2 * NCH * E], f8, kind="ExternalInput").ap()
    out_d = nc.dram_tensor("out", [OUT_ROWS, E], bf16, kind="ExternalOutput").ap()

    L4 = _L[2]
    l4b = L4["mh_base"]
    l4np = L4["npairs"]

    with tile.TileContext(nc) as tc, ExitStack() as ctx:
        cpool = ctx.enter_context(tc.tile_pool(name="const", bufs=1))
        pspool = ctx.enter_context(
            tc.tile_pool(name="ps", bufs=7, space=bass.MemorySpace.PSUM)
        )
        wpool = ctx.enter_context(
            tc.tile_pool(name="wps", bufs=1, space=bass.MemorySpace.PSUM)
        )
        spool = ctx.enter_context(tc.tile_pool(name="stage", bufs=1))

        tb_t = cpool.tile([128, 2 * NCH * E], f8, tag="tb")
        mh_t = cpool.tile([128, MH_TOTAL + DRV_TOTAL], f8, tag="mh")
        # [p, 2, x] view pairing L4's packed v region with the derived region
        l4_pair_view = mh_t[:, l4b : l4b + 2 * DRV_TOTAL].rearrange(
            "p (two x) -> p two x", two=2
        )

        def emit_derived(t0, ptt):
            """is_ge(v, 2) for piece ttiles [t0, t0+ptt): DVE pairs 0..2, Pool 3..5."""
            blk = l4np * 128
            src = mh_t[:, l4b + t0 * blk : l4b + (t0 + ptt) * blk].rearrange(
                "p (tt x) -> p tt x", tt=ptt
            )
            dst = mh_t[
                :, DRV_BASE + t0 * blk : DRV_BASE + (t0 + ptt) * blk
            ].rearrange("p (tt x) -> p tt x", tt=ptt)
            cut = DVE_PAIRS * 128
            nc.vector.tensor_scalar(
                dst[:, :, :cut], src[:, :, :cut], 2.0, None,
                op0=mybir.AluOpType.is_ge,
            )
            nc.gpsimd.tensor_scalar(
                dst[:, :, cut:], src[:, :, cut:], 2.0, None,
                op0=mybir.AluOpType.is_ge,
            )

        # loads in processing order (SP queue); hi-plane table before a layer's
        # first mh piece, lo-plane deferred TBL_AFTER pieces (per-ttile matmuls
        # run all-hi then all-lo); derived ops chase each L4 piece
        # PE warmup: dummy DoubleRow matmuls on zeroed scratch keep the PE
        # continuously busy from t~0 so the p-state ramp completes before the
        # first real matmul (results discarded)
        if NWARM:
            # fused scratch, one DVE memset gating the first dummy
            wsc = cpool.tile([128, 768], f8, tag="wsc")
            nc.vector.memset(wsc[:], 0.0)
            wps = wpool.tile([128, E], f32, tag="wps")
            for i in range(NWARM):
                nc.tensor.matmul(
                    wps[:],
                    wsc[:, 0:256].rearrange("p (two m) -> p two m", two=2),
                    wsc[:, 256:768].rearrange("p (two e) -> p two e", two=2),
                    start=(i == 0),
                    stop=(i == NWARM - 1),
                    perf_mode=DR,
                )

        tb_loaded = set()
        for li, g0, gn, pieces, stores in SCHEDULE:
            d = _L[li]
            fresh = li not in tb_loaded and pieces
            if fresh and TBH_AFTER.get(li, -1) < 0:
                tb_loaded.add(li)
                ca = 2 * d["cb"] * E
                n = d["nch"] * E if li in TBL_AFTER else 2 * d["nch"] * E
                nc.sync.dma_start(tb_t[:, ca : ca + n], tb_d[:, ca : ca + n])
            t0 = g0
            for pi, ptt in enumerate(pieces):
                a = d["mh_base"] + t0 * d["nblk"] * 128
                bnd = a + ptt * d["nblk"] * 128
                nc.sync.dma_start(mh_t[:, a:bnd], mh_d[:, a:bnd])
                if d["packed"]:
                    emit_derived(t0, ptt)
                t0 += ptt
                if fresh and pi + 1 == TBH_AFTER.get(li, -1):
                    tb_loaded.add(li)
                    ca = 2 * d["cb"] * E
                    nc.sync.dma_start(
                        tb_t[:, ca : ca + d["nch"] * E],
                        tb_d[:, ca : ca + d["nch"] * E],
                    )
                if fresh and pi + 1 == TBL_AFTER.get(li, -1):
                    ca = (2 * d["cb"] + d["nch"]) * E
                    nc.sync.dma_start(
                        tb_t[:, ca : ca + d["nch"] * E],
                        tb_d[:, ca : ca + d["nch"] * E],
                    )

        # compute
        ev = 0
        st = 0
        for li, g0, gn, pieces, stores in SCHEDULE:
            d = _L[li]
            inv_s = inv_scales[li]
            stage = spool.tile([128, gn * E], bf16, tag=f"st{li}g{g0}")
            for ti in range(gn):
                tt = g0 + ti
                ps = pspool.tile([128, E], f32, tag="ps")
                nmm = 2 * d["npairs"]
                i = 0
                for hl in range(2):
                    for q in range(d["npairs"]):
                        if d["packed"]:
                            off = (tt * d["npairs"] + q) * 128
                            mh_ap = l4_pair_view[:, :, off : off + 128]
                        else:
                            ma = d["mh_base"] + (tt * d["nch"] + 2 * q) * 128
                            mh_ap = mh_t[:, ma : ma + 256].rearrange(
                                "p (two m) -> p two m", two=2
                            )
                        ta = (2 * d["cb"] + hl * d["nch"] + 2 * q) * E
                        nc.tensor.matmul(
                            ps[:],
                            mh_ap,
                            tb_t[:, ta : ta + 2 * E].rearrange(
                                "p (two e) -> p two e", two=2
                            ),
                            start=(i == 0),
                            stop=(i == nmm - 1),
                            perf_mode=DR,
                        )
                        i += 1
                dst = stage[:, ti * E : (ti + 1) * E]
                # B is computed last: strictly alternate so its evicts overlap
                eng = ("dve", "act")[ev % 2] if li == 0 else EVICT_PAT[ev % len(EVICT_PAT)]
                ev += 1
                if eng == "dve":
                    nc.vector.tensor_scalar(
                        dst, ps[:], inv_s, None, op0=mybir.AluOpType.mult
                    )
                else:
                    nc.scalar.activation(dst, ps[:], A.Copy, scale=inv_s)
            s0 = 0
            for sn in stores:
                r0 = d["out_row0"] + (g0 + s0) * 128
                seng = nc.scalar if st % 2 == 0 else nc.sync
                st += 1
                seng.dma_start(
                    out_d[r0 : r0 + sn * 128, :].rearrange(
                        "(a p) e -> p a e", p=128
                    ),
                    stage[:, s0 * E : (s0 + sn) * E].rearrange(
                        "p (a e) -> p a e", e=E
                    ),
                )
                s0 += sn

    nc.compile()
    _CACHE[key] = nc
    return nc


def kernel(**inputs):
    from concourse.bass_utils import run_bass_kernel_spmd

    value = np.asarray(inputs["value"], np.int64)
    depth = np.asarray(inputs["depth"], np.int64)
    position = np.asarray(inputs["position"], np.int64)
    params = {
        k: np.asarray(v, np.float32)
        for k, v in inputs.items()
        if "emb" in k or "conv" in k
    }

    tabs = _build_tables(params)

    # B per-core rows 0..71 (l0+l1 per-token sums) + l2 table + dep uniques
    dep2_uniq = np.unique(depth[:, 72:584])
    dep2_rows = np.asarray(params["dep_emb_2"], np.float32)[dep2_uniq]
    assert 264 + len(dep2_uniq) <= 512
    b_rows_percore = []
    for b in range(BATCH):
        r01 = np.zeros((72, E), np.float32)
        for l, (lo, hi) in ((0, (0, 8)), (1, (8, 72))):
            v = value[b, lo:hi]
            p = position[b, lo:hi]
            dd = depth[b, lo:hi]
            pe = np.asarray(params[f"pos_emb_{l}"], np.float32)
            r01[lo:hi] = (
                np.asarray(params[f"val_emb_{l}"], np.float32)[v]
                + pe[0][p[:, 0]]
                + pe[1][p[:, 1]]
                + pe[2][p[:, 2]]
                + np.asarray(params[f"dep_emb_{l}"], np.float32)[dd]
            )
        b_rows_percore.append(np.concatenate([r01, tabs["B_l2"], dep2_rows], 0))

    # per-layer scales (shared across cores -> compiled immediates)
    S = [
        min(_layer_scale(r, _L[0]) for r in b_rows_percore),
        _layer_scale(tabs["L3"], _L[1]),
        _layer_scale(tabs["L4"], _L[2]),
    ]
    inv_s = tuple(float(1.0 / s) for s in S)

    nc = _get_nc(inv_s)

    # table tensor: per layer [hi chunks | lo chunks] contiguous (1 DMA/layer)
    tb_shared = np.zeros((128, 2 * NCH * E), F8)
    for li, name in ((1, "L3"), (2, "L4")):
        d = _L[li]
        hi, lo = _pack_layer_tb(tabs[name], d, S[li])
        ca = 2 * d["cb"] * E
        tb_shared[:, ca : ca + d["nch"] * E] = hi
        tb_shared[:, ca + d["nch"] * E : ca + 2 * d["nch"] * E] = lo

    in_maps = []
    for b in range(BATCH):
        tb = tb_shared.copy()
        hi, lo = _pack_layer_tb(b_rows_percore[b], _L[0], S[0])
        tb[:, : _L[0]["nch"] * E] = hi
        tb[:, _L[0]["nch"] * E : 2 * _L[0]["nch"] * E] = lo
        mh = _build_mh(value, depth, position, b, dep2_uniq).view(F8)
        in_maps.append({"mh": mh, "tb": tb})

    res = run_bass_kernel_spmd(nc, in_maps, list(range(BATCH)))
    outs = []
    for b in range(BATCH):
        o = np.asarray(res.results[b]["out"]).astype(np.float32)
        outs.append(np.concatenate([o[0:584], o[640:1664], o[1664:5760]], 0))
    return np.stack(outs)
